# revision 18
# baseline (speedup 1.0000x reference)
"""ChannelInteractionAttention on 8 TRN2 NeuronCores (Bass/Tile).

Math (exact algebraic simplification of the reference):
  The channel affinity x_c x_c^T is symmetric, so concat(affinity, affinity^T)
  pools to x_c[i] * concat(p, p) with p = window-4 mean of x_c.  The whole MLP
  gate collapses to, per sample:
      u[h] = sum_c fc1_w[h, c] * x_c[c] + fc1_b[h]
      v[h] = sum_k (fc1_w[h, 512+k] + fc1_w[h, 640+k]) * p[k]
      s[i, h] = relu(u[h] + x_c[i] * v[h])
      a[i] = sigmoid(sum_h fc2_w[0, h] * s[i, h] + fc2_b[0])
      out[i, :, :] = x[i, :, :] * a[i]
  u and v are matvecs against x_sum (spatial sums); the 1/HW normalisations
  and the window-4 pooling matrix are folded into host-precomputed weights.

Sharding: data-parallel over batch B=32 -> 4 samples per core; the tiny
folded weights are replicated.  Each core streams its 32 MiB shard through
SBUF once (read), computes the gate on-chip, scales in place and writes the
32 MiB result: HBM traffic is the 2x compulsory minimum.
"""

import sys

if "/opt/trn_rl_repo" not in sys.path:
    sys.path.insert(0, "/opt/trn_rl_repo")

import numpy as np

import concourse.bacc as bacc
import concourse.tile as tile_mod
from concourse import mybir
from concourse.bass_utils import run_bass_kernel_spmd

N_CORES = 8
B, C, H, W = 32, 512, 64, 64
HW = H * W          # 4096
BPC = B // N_CORES  # samples per core = 4
NG = C // 128       # channel groups of 128 = 4
HID = 64            # fc1 rows
F32 = mybir.dt.float32

XT_BUFS = 12        # [128, 4096] f32 tiles: 16 KiB/partition each

LAST_RESULT = None  # BassKernelResults of the most recent run (for test.py)


def _build_nc():
    # Bacc (not raw Bass): its compile() pass legalizes Tile's multi-wait
    # instructions, which this walrus build otherwise rejects.
    nc = bacc.Bacc("TRN2", target_bir_lowering=False, debug=False,
                   num_devices=N_CORES)
    x_d = nc.dram_tensor("x", [BPC, C, HW], F32, kind="ExternalInput")
    wu_d = nc.dram_tensor("wu", [128, NG * HID], F32, kind="ExternalInput")
    wv_d = nc.dram_tensor("wv", [128, NG * HID], F32, kind="ExternalInput")
    w2_d = nc.dram_tensor("w2", [HID, 1], F32, kind="ExternalInput")
    b1_d = nc.dram_tensor("b1", [HID, 1], F32, kind="ExternalInput")
    b2_d = nc.dram_tensor("b2", [128, 1], F32, kind="ExternalInput")
    id_d = nc.dram_tensor("ident", [128, 128], F32, kind="ExternalInput")
    out_d = nc.dram_tensor("out", [BPC, C, HW], F32, kind="ExternalOutput")

    Relu = mybir.ActivationFunctionType.Relu
    Sigmoid = mybir.ActivationFunctionType.Sigmoid
    Copy = mybir.ActivationFunctionType.Copy

    with tile_mod.TileContext(nc) as tc:
        with (
            tc.tile_pool(name="xt", bufs=XT_BUFS) as p_xt,
            tc.tile_pool(name="small", bufs=2) as p_small,
            tc.tile_pool(name="consts", bufs=1) as p_const,
            tc.tile_pool(name="ps_row", bufs=1, space="PSUM") as pp_row,
            tc.tile_pool(name="ps_bc", bufs=1, space="PSUM") as pp_bc,
            tc.tile_pool(name="ps_uv", bufs=1, space="PSUM") as pp_uv,
            tc.tile_pool(name="ps_a", bufs=1, space="PSUM") as pp_a,
        ):
            from concourse.tile import add_dep_helper

            def load_sample(b):
                xts, insts = [], []
                for g in range(NG):
                    xt = p_xt.tile([128, HW], F32, tag="xt")
                    ins = nc.sync.dma_start(
                        out=xt[:], in_=x_d[b, 128 * g : 128 * (g + 1), :]
                    )
                    xts.append(xt)
                    insts.append(ins)
                return xts, insts

            # loads run two samples ahead of the gate/store stream so the SP
            # queue never starves while a gate semaphore blocks store issue
            pending = {0: load_sample(0)}

            # consts go via the gpsimd SWDGE queue so the SP ring is pure x-DMA
            ident_sb = p_const.tile([128, 128], F32, tag="ident")
            nc.gpsimd.dma_start(out=ident_sb[:], in_=id_d[:])
            wu_sb = p_const.tile([128, NG * HID], F32, tag="wu")
            nc.gpsimd.dma_start(out=wu_sb[:], in_=wu_d[:])
            wv_sb = p_const.tile([128, NG * HID], F32, tag="wv")
            nc.gpsimd.dma_start(out=wv_sb[:], in_=wv_d[:])
            w2_sb = p_const.tile([HID, 1], F32, tag="w2")
            nc.gpsimd.dma_start(out=w2_sb[:], in_=w2_d[:])
            b1_sb = p_const.tile([HID, 1], F32, tag="b1")
            nc.gpsimd.dma_start(out=b1_sb[:], in_=b1_d[:])
            b2_sb = p_const.tile([128, 1], F32, tag="b2")
            nc.gpsimd.dma_start(out=b2_sb[:], in_=b2_d[:])
            ones_sb = p_const.tile([1, HID], F32, tag="ones")
            nc.vector.memset(ones_sb[:], 1.0)

            for b in range(BPC):
                if b + 1 < BPC:
                    pending[b + 1] = load_sample(b + 1)
                xts, _ = pending.pop(b)
                next_loads = pending[b + 1][1] if b + 1 < BPC else None
                xsum = p_small.tile([128, NG], F32, tag="xsum")
                for g in range(NG):
                    nc.vector.reduce_sum(
                        out=xsum[:, g : g + 1], in_=xts[g][:],
                        axis=mybir.AxisListType.X,
                    )

                # x_sum columns -> one [1, 512] row (4 tiny PE transposes)
                xrow_ps = pp_row.tile([1, C], F32, tag="xrow")
                for g in range(NG):
                    nc.tensor.transpose(
                        out=xrow_ps[0:1, 128 * g : 128 * (g + 1)],
                        in_=xsum[:, g : g + 1],
                        identity=ident_sb[:],
                    )
                xrow_sb = p_small.tile([1, C], F32, tag="xrowsb")
                nc.vector.tensor_copy(out=xrow_sb[:], in_=xrow_ps[:])

                # broadcast the row over HID partitions: ones[1,HID]^T @ xrow[1,C]
                xbc_ps = pp_bc.tile([HID, C], F32, tag="xbc")
                nc.tensor.matmul(
                    out=xbc_ps[:], lhsT=ones_sb[:], rhs=xrow_sb[:],
                    start=True, stop=True,
                )

                # u = Wu^T @ xsum, v = Wv^T @ xsum   (accumulate over groups)
                u_ps = pp_uv.tile([HID, 1], F32, tag="u")
                v_ps = pp_uv.tile([HID, 1], F32, tag="v")
                for g in range(NG):
                    nc.tensor.matmul(
                        out=u_ps[:], lhsT=wu_sb[:, HID * g : HID * (g + 1)],
                        rhs=xsum[:, g : g + 1],
                        start=(g == 0), stop=(g == NG - 1),
                    )
                for g in range(NG):
                    nc.tensor.matmul(
                        out=v_ps[:], lhsT=wv_sb[:, HID * g : HID * (g + 1)],
                        rhs=xsum[:, g : g + 1],
                        start=(g == 0), stop=(g == NG - 1),
                    )
                ub_sb = p_small.tile([HID, 1], F32, tag="ub")
                nc.vector.tensor_add(out=ub_sb[:], in0=u_ps[:], in1=b1_sb[:])
                v_sb = p_small.tile([HID, 1], F32, tag="vsb")
                nc.vector.tensor_copy(out=v_sb[:], in_=v_ps[:])

                # s^T[h, i] = relu(v[h] * xsum[i] + u[h] + b1[h])
                sT_sb = p_small.tile([HID, C], F32, tag="sT")
                nc.scalar.activation(
                    out=sT_sb[:], in_=xbc_ps[:], func=Relu,
                    bias=ub_sb[:], scale=v_sb[:],
                )

                # a_pre[i] = sum_h s^T[h, i] * w2[h]  (4 matmuls, k=HID)
                a_ps = pp_a.tile([128, NG], F32, tag="aps")
                for g in range(NG):
                    nc.tensor.matmul(
                        out=a_ps[:, g : g + 1],
                        lhsT=sT_sb[:, 128 * g : 128 * (g + 1)],
                        rhs=w2_sb[:],
                        start=True, stop=True,
                    )
                a_sb = p_small.tile([128, NG], F32, tag="asb")
                nc.scalar.activation(
                    out=a_sb[:], in_=a_ps[:], func=Sigmoid,
                    bias=b2_sb[:], scale=1.0,
                )

                # scale in place and store.  Stores issue from ACT (its own
                # HWDGE ring) so the SP load stream never blocks on gate sems.
                # Last sample: alternate DVE/ACT scales to compress the tail
                # (safe there — no later reduces to serialize behind on DVE).
                last = b == BPC - 1
                for g in range(NG):
                    if last and g % 2 == 0:
                        nc.vector.tensor_scalar_mul(
                            out=xts[g][:], in0=xts[g][:],
                            scalar1=a_sb[:, g : g + 1],
                        )
                    else:
                        nc.scalar.activation(
                            out=xts[g][:], in_=xts[g][:], func=Copy,
                            scale=a_sb[:, g : g + 1],
                        )
                    st = nc.sync.dma_start(
                        out=out_d[b, 128 * g : 128 * (g + 1), :], in_=xts[g][:]
                    )
                    if b == BPC - 2 and next_loads is not None:
                        # keep the second-to-last sample's stores behind the
                        # last sample's loads on the SP ring: that pending
                        # store work then covers the final gate latency
                        # instead of draining early
                        add_dep_helper(
                            st.ins, next_loads[-1].ins,
                            reason="penultimate stores cover final gate",
                        )
    nc.compile()
    return nc


_NC_CACHE = None


def _get_nc():
    global _NC_CACHE
    if _NC_CACHE is None:
        _NC_CACHE = _build_nc()
    return _NC_CACHE


def kernel(x, fc1_w, fc1_b, fc2_w, fc2_b):
    global LAST_RESULT
    x = np.ascontiguousarray(np.asarray(x, dtype=np.float32))
    fc1_w = np.asarray(fc1_w, dtype=np.float32)
    fc1_b = np.asarray(fc1_b, dtype=np.float32)
    fc2_w = np.asarray(fc2_w, dtype=np.float32)
    fc2_b = np.asarray(fc2_b, dtype=np.float32)

    # fold 1/HW (mean), the window-4 pooling and the affinity symmetry into
    # the fc1 weights; keep them as [k=channel-chunk, m] lhsT layouts.
    wu_full = fc1_w[:, :C].T / float(HW)                       # [C, HID]
    w1bc = fc1_w[:, C : C + 128] + fc1_w[:, C + 128 : C + 256]  # [HID, 128]
    wv_full = np.repeat(w1bc.T, 4, axis=0) / (4.0 * HW * HW)   # [C, HID]
    wu = np.ascontiguousarray(
        wu_full.reshape(NG, 128, HID).transpose(1, 0, 2).reshape(128, NG * HID),
        dtype=np.float32,
    )
    wv = np.ascontiguousarray(
        wv_full.reshape(NG, 128, HID).transpose(1, 0, 2).reshape(128, NG * HID),
        dtype=np.float32,
    )
    w2 = np.ascontiguousarray(fc2_w.reshape(1, HID).T, dtype=np.float32)  # [HID,1]
    b1 = np.ascontiguousarray(fc1_b.reshape(HID, 1), dtype=np.float32)
    b2 = np.full((128, 1), float(fc2_b.reshape(-1)[0]), dtype=np.float32)
    ident = np.eye(128, dtype=np.float32)

    xs = x.reshape(N_CORES, BPC, C, HW)
    in_maps = [
        {
            "x": xs[i],
            "wu": wu,
            "wv": wv,
            "w2": w2,
            "b1": b1,
            "b2": b2,
            "ident": ident,
        }
        for i in range(N_CORES)
    ]

    nc = _get_nc()
    res = run_bass_kernel_spmd(nc, in_maps, list(range(N_CORES)))
    LAST_RESULT = res
    out = np.empty((B, C, H, W), dtype=np.float32)
    for i in range(N_CORES):
        out[i * BPC : (i + 1) * BPC] = res.results[i]["out"].reshape(BPC, C, H, W)
    return out


# revision 19
# speedup vs baseline: 1.0194x; 1.0194x over previous
"""ChannelInteractionAttention on 8 TRN2 NeuronCores (Bass/Tile).

Math (exact algebraic simplification of the reference):
  The channel affinity x_c x_c^T is symmetric, so concat(affinity, affinity^T)
  pools to x_c[i] * concat(p, p) with p = window-4 mean of x_c.  The whole MLP
  gate collapses to, per sample:
      u[h] = sum_c fc1_w[h, c] * x_c[c] + fc1_b[h]
      v[h] = sum_k (fc1_w[h, 512+k] + fc1_w[h, 640+k]) * p[k]
      s[i, h] = relu(u[h] + x_c[i] * v[h])
      a[i] = sigmoid(sum_h fc2_w[0, h] * s[i, h] + fc2_b[0])
      out[i, :, :] = x[i, :, :] * a[i]
  u and v are matvecs against x_sum (spatial sums); the 1/HW normalisations
  and the window-4 pooling matrix are folded into host-precomputed weights.

Sharding: data-parallel over batch B=32 -> 4 samples per core; the tiny
folded weights are replicated.  Each core streams its 32 MiB shard through
SBUF once (read), computes the gate on-chip, scales in place and writes the
32 MiB result: HBM traffic is the 2x compulsory minimum.

Pipeline: loads run one sample ahead of the gate/store stream; the last
sample's final channel group is loaded in 512 KiB chunks with incremental
reduction, and the whole gate runs per-group, so the tail (last gate -> first
store) is short.
"""

import sys

if "/opt/trn_rl_repo" not in sys.path:
    sys.path.insert(0, "/opt/trn_rl_repo")

import numpy as np

import concourse.bacc as bacc
import concourse.tile as tile_mod
from concourse import mybir
from concourse.bass_utils import run_bass_kernel_spmd

N_CORES = 8
B, C, H, W = 32, 512, 64, 64
HW = H * W          # 4096
BPC = B // N_CORES  # samples per core = 4
NG = C // 128       # channel groups of 128 = 4
HID = 64            # fc1 rows
NCH = 4             # chunks for the tail group's incremental reduce
F32 = mybir.dt.float32

XT_BUFS = 11        # [128, 4096] f32 tiles: 16 KiB/partition each

LAST_RESULT = None  # BassKernelResults of the most recent run (for test.py)


def _build_nc():
    # Bacc (not raw Bass): its compile() pass legalizes Tile's multi-wait
    # instructions, which this walrus build otherwise rejects.
    nc = bacc.Bacc("TRN2", target_bir_lowering=False, debug=False,
                   num_devices=N_CORES)
    x_d = nc.dram_tensor("x", [BPC, C, HW], F32, kind="ExternalInput")
    wu_d = nc.dram_tensor("wu", [128, NG * HID], F32, kind="ExternalInput")
    wv_d = nc.dram_tensor("wv", [128, NG * HID], F32, kind="ExternalInput")
    w2_d = nc.dram_tensor("w2", [HID, 1], F32, kind="ExternalInput")
    b1_d = nc.dram_tensor("b1", [HID, 1], F32, kind="ExternalInput")
    b2_d = nc.dram_tensor("b2", [128, 1], F32, kind="ExternalInput")
    id_d = nc.dram_tensor("ident", [128, 128], F32, kind="ExternalInput")
    out_d = nc.dram_tensor("out", [BPC, C, HW], F32, kind="ExternalOutput")

    Relu = mybir.ActivationFunctionType.Relu
    Sigmoid = mybir.ActivationFunctionType.Sigmoid
    Copy = mybir.ActivationFunctionType.Copy
    CW = HW // NCH

    with tile_mod.TileContext(nc) as tc:
        with (
            tc.tile_pool(name="xt", bufs=XT_BUFS) as p_xt,
            tc.tile_pool(name="xtc", bufs=NCH) as p_chunk,
            tc.tile_pool(name="small", bufs=2) as p_small,
            tc.tile_pool(name="consts", bufs=1) as p_const,
            tc.tile_pool(name="ps_row", bufs=1, space="PSUM") as pp_row,
            tc.tile_pool(name="ps_bc", bufs=1, space="PSUM") as pp_bc,
            tc.tile_pool(name="ps_uv", bufs=1, space="PSUM") as pp_uv,
            tc.tile_pool(name="ps_a", bufs=1, space="PSUM") as pp_a,
        ):
            def load_sample(b):
                """Emit loads + incremental reduces; returns (xsum, tiles).

                tiles[g] is either a full [128, HW] tile, or (for the very
                last group of the last sample) a list of NCH chunk tiles —
                chunked so the final reduce finishes right after the last
                bytes land instead of a full 4096-wide reduce later.
                """
                xsum = p_small.tile([128, NG], F32, tag="xsum")
                tiles = []
                for g in range(NG):
                    if b == BPC - 1 and g == NG - 1:
                        tmp = p_small.tile([128, NCH], F32, tag="redtmp")
                        chunks = []
                        for cix in range(NCH):
                            xc = p_chunk.tile([128, CW], F32, tag="xtc")
                            nc.sync.dma_start(
                                out=xc[:],
                                in_=x_d[b, 128 * g : 128 * (g + 1),
                                        CW * cix : CW * (cix + 1)],
                            )
                            nc.vector.reduce_sum(
                                out=tmp[:, cix : cix + 1], in_=xc[:],
                                axis=mybir.AxisListType.X,
                            )
                            chunks.append(xc)
                        nc.vector.reduce_sum(
                            out=xsum[:, g : g + 1], in_=tmp[:],
                            axis=mybir.AxisListType.X,
                        )
                        tiles.append(chunks)
                    else:
                        xt = p_xt.tile([128, HW], F32, tag="xt")
                        nc.sync.dma_start(
                            out=xt[:], in_=x_d[b, 128 * g : 128 * (g + 1), :]
                        )
                        nc.vector.reduce_sum(
                            out=xsum[:, g : g + 1], in_=xt[:],
                            axis=mybir.AxisListType.X,
                        )
                        tiles.append(xt)
                return xsum, tiles

            # loads + reduces run one sample ahead of the gate/store stream
            pending = {0: load_sample(0)}

            # consts go via the gpsimd SWDGE queue so the SP ring is pure x-DMA
            ident_sb = p_const.tile([128, 128], F32, tag="ident")
            nc.gpsimd.dma_start(out=ident_sb[:], in_=id_d[:])
            wu_sb = p_const.tile([128, NG * HID], F32, tag="wu")
            nc.gpsimd.dma_start(out=wu_sb[:], in_=wu_d[:])
            wv_sb = p_const.tile([128, NG * HID], F32, tag="wv")
            nc.gpsimd.dma_start(out=wv_sb[:], in_=wv_d[:])
            w2_sb = p_const.tile([HID, 1], F32, tag="w2")
            nc.gpsimd.dma_start(out=w2_sb[:], in_=w2_d[:])
            b1_sb = p_const.tile([HID, 1], F32, tag="b1")
            nc.gpsimd.dma_start(out=b1_sb[:], in_=b1_d[:])
            b2_sb = p_const.tile([128, 1], F32, tag="b2")
            nc.gpsimd.dma_start(out=b2_sb[:], in_=b2_d[:])
            ones_sb = p_const.tile([1, HID], F32, tag="ones")
            nc.vector.memset(ones_sb[:], 1.0)

            for b in range(BPC):
                if b + 1 < BPC:
                    pending[b + 1] = load_sample(b + 1)
                xsum, tiles = pending.pop(b)
                last = b == BPC - 1

                # u = Wu^T @ xsum, v = Wv^T @ xsum   (accumulate over groups)
                u_ps = pp_uv.tile([HID, 1], F32, tag="u")
                v_ps = pp_uv.tile([HID, 1], F32, tag="v")
                for g in range(NG):
                    nc.tensor.matmul(
                        out=u_ps[:], lhsT=wu_sb[:, HID * g : HID * (g + 1)],
                        rhs=xsum[:, g : g + 1],
                        start=(g == 0), stop=(g == NG - 1),
                    )
                for g in range(NG):
                    nc.tensor.matmul(
                        out=v_ps[:], lhsT=wv_sb[:, HID * g : HID * (g + 1)],
                        rhs=xsum[:, g : g + 1],
                        start=(g == 0), stop=(g == NG - 1),
                    )
                ub_sb = p_small.tile([HID, 1], F32, tag="ub")
                nc.vector.tensor_add(out=ub_sb[:], in0=u_ps[:], in1=b1_sb[:])
                v_sb = p_small.tile([HID, 1], F32, tag="vsb")
                nc.vector.tensor_copy(out=v_sb[:], in_=v_ps[:])

                # per-group gate pipeline: each group's scale factor becomes
                # ready as early as possible (short last-gate tail)
                xrow_ps = pp_row.tile([1, C], F32, tag="xrow")
                xrow_sb = p_small.tile([1, C], F32, tag="xrowsb")
                xbc_ps = pp_bc.tile([HID, C], F32, tag="xbc")
                sT_sb = p_small.tile([HID, C], F32, tag="sT")
                a_ps = pp_a.tile([128, NG], F32, tag="aps")
                a_sb = p_small.tile([128, NG], F32, tag="asb")
                for g in range(NG):
                    sl = slice(128 * g, 128 * (g + 1))
                    # x_sum column -> row piece (tiny PE transpose), to SBUF
                    nc.tensor.transpose(
                        out=xrow_ps[0:1, sl], in_=xsum[:, g : g + 1],
                        identity=ident_sb[:],
                    )
                    nc.vector.tensor_copy(
                        out=xrow_sb[0:1, sl], in_=xrow_ps[0:1, sl]
                    )
                    # broadcast over HID partitions: ones[1,HID]^T @ row piece
                    nc.tensor.matmul(
                        out=xbc_ps[:, sl], lhsT=ones_sb[:],
                        rhs=xrow_sb[0:1, sl], start=True, stop=True,
                    )
                    # s^T[h, i] = relu(v[h] * xsum[i] + u[h] + b1[h])
                    nc.scalar.activation(
                        out=sT_sb[:, sl], in_=xbc_ps[:, sl], func=Relu,
                        bias=ub_sb[:], scale=v_sb[:],
                    )
                    # a_pre[i] = sum_h s^T[h, i] * w2[h]
                    nc.tensor.matmul(
                        out=a_ps[:, g : g + 1], lhsT=sT_sb[:, sl],
                        rhs=w2_sb[:], start=True, stop=True,
                    )
                    nc.scalar.activation(
                        out=a_sb[:, g : g + 1], in_=a_ps[:, g : g + 1],
                        func=Sigmoid, bias=b2_sb[:], scale=1.0,
                    )

                # scale in place and store.  Last sample alternates DVE/ACT
                # scales to compress the tail (safe there — no later reduces
                # to serialize behind on the DVE FIFO).
                def scale(tile_ap, a_col, on_dve):
                    if on_dve:
                        nc.vector.tensor_scalar_mul(
                            out=tile_ap, in0=tile_ap, scalar1=a_col
                        )
                    else:
                        nc.scalar.activation(
                            out=tile_ap, in_=tile_ap, func=Copy, scale=a_col
                        )

                for g in range(NG):
                    a_col = a_sb[:, g : g + 1]
                    if isinstance(tiles[g], list):
                        for cix, xc in enumerate(tiles[g]):
                            scale(xc[:], a_col, last and cix % 2 == 0)
                            nc.sync.dma_start(
                                out=x_dst(out_d, b, g, cix), in_=xc[:]
                            )
                    else:
                        scale(tiles[g][:], a_col, last and g % 2 == 0)
                        nc.sync.dma_start(
                            out=out_d[b, 128 * g : 128 * (g + 1), :],
                            in_=tiles[g][:],
                        )
    nc.compile()
    return nc


def x_dst(out_d, b, g, cix):
    CW = HW // NCH
    return out_d[b, 128 * g : 128 * (g + 1), CW * cix : CW * (cix + 1)]


_NC_CACHE = None


def _get_nc():
    global _NC_CACHE
    if _NC_CACHE is None:
        _NC_CACHE = _build_nc()
    return _NC_CACHE


def kernel(x, fc1_w, fc1_b, fc2_w, fc2_b):
    global LAST_RESULT
    x = np.ascontiguousarray(np.asarray(x, dtype=np.float32))
    fc1_w = np.asarray(fc1_w, dtype=np.float32)
    fc1_b = np.asarray(fc1_b, dtype=np.float32)
    fc2_w = np.asarray(fc2_w, dtype=np.float32)
    fc2_b = np.asarray(fc2_b, dtype=np.float32)

    # fold 1/HW (mean), the window-4 pooling and the affinity symmetry into
    # the fc1 weights; keep them as [k=channel-chunk, m] lhsT layouts.
    wu_full = fc1_w[:, :C].T / float(HW)                       # [C, HID]
    w1bc = fc1_w[:, C : C + 128] + fc1_w[:, C + 128 : C + 256]  # [HID, 128]
    wv_full = np.repeat(w1bc.T, 4, axis=0) / (4.0 * HW * HW)   # [C, HID]
    wu = np.ascontiguousarray(
        wu_full.reshape(NG, 128, HID).transpose(1, 0, 2).reshape(128, NG * HID),
        dtype=np.float32,
    )
    wv = np.ascontiguousarray(
        wv_full.reshape(NG, 128, HID).transpose(1, 0, 2).reshape(128, NG * HID),
        dtype=np.float32,
    )
    w2 = np.ascontiguousarray(fc2_w.reshape(1, HID).T, dtype=np.float32)  # [HID,1]
    b1 = np.ascontiguousarray(fc1_b.reshape(HID, 1), dtype=np.float32)
    b2 = np.full((128, 1), float(fc2_b.reshape(-1)[0]), dtype=np.float32)
    ident = np.eye(128, dtype=np.float32)

    xs = x.reshape(N_CORES, BPC, C, HW)
    in_maps = [
        {
            "x": xs[i],
            "wu": wu,
            "wv": wv,
            "w2": w2,
            "b1": b1,
            "b2": b2,
            "ident": ident,
        }
        for i in range(N_CORES)
    ]

    nc = _get_nc()
    res = run_bass_kernel_spmd(nc, in_maps, list(range(N_CORES)))
    LAST_RESULT = res
    out = np.empty((B, C, H, W), dtype=np.float32)
    for i in range(N_CORES):
        out[i * BPC : (i + 1) * BPC] = res.results[i]["out"].reshape(BPC, C, H, W)
    return out


# revision 20
# speedup vs baseline: 1.2638x; 1.2398x over previous
"""ChannelInteractionAttention on 8 TRN2 NeuronCores (Bass/Tile).

Math (exact algebraic simplification of the reference):
  The channel affinity x_c x_c^T is symmetric, so concat(affinity, affinity^T)
  pools to x_c[i] * concat(p, p) with p = window-4 mean of x_c.  The whole MLP
  gate collapses to, per sample:
      u[h] = sum_c fc1_w[h, c] * x_c[c] + fc1_b[h]
      v[h] = sum_k (fc1_w[h, 512+k] + fc1_w[h, 640+k]) * p[k]
      s[i, h] = relu(u[h] + x_c[i] * v[h])
      a[i] = sigmoid(sum_h fc2_w[0, h] * s[i, h] + fc2_b[0])
      out[i, :, :] = x[i, :, :] * a[i]
  u and v are matvecs against x_sum (spatial sums); the 1/HW normalisations
  and the window-4 pooling matrix are folded into host-precomputed weights.

Sharding: data-parallel over batch B=32 -> 4 samples per core; the tiny
folded weights are replicated.  Each core streams its 32 MiB shard through
SBUF once (read), computes the gate on-chip, scales in place and writes the
32 MiB result: HBM traffic is the 2x compulsory minimum.

Pipeline: loads run one sample ahead of the gate/store stream; the last
sample's final channel group is loaded in 512 KiB chunks with incremental
reduction, and the whole gate runs per-group, so the tail (last gate -> first
store) is short.
"""

import sys

if "/opt/trn_rl_repo" not in sys.path:
    sys.path.insert(0, "/opt/trn_rl_repo")

import numpy as np

import concourse.bacc as bacc
import concourse.tile as tile_mod
from concourse import mybir
from concourse.bass_utils import run_bass_kernel_spmd

N_CORES = 8
B, C, H, W = 32, 512, 64, 64
HW = H * W          # 4096
BPC = B // N_CORES  # samples per core = 4
NG = C // 128       # channel groups of 128 = 4
HID = 64            # fc1 rows
NCH = 4             # chunks for the tail group's incremental reduce
F32 = mybir.dt.float32

XT_BUFS = 11        # [128, 4096] f32 tiles: 16 KiB/partition each

LAST_RESULT = None  # BassKernelResults of the most recent run (for test.py)


def _build_nc():
    # Bacc (not raw Bass): its compile() pass legalizes Tile's multi-wait
    # instructions, which this walrus build otherwise rejects.
    nc = bacc.Bacc("TRN2", target_bir_lowering=False, debug=False,
                   num_devices=N_CORES)
    x_d = nc.dram_tensor("x", [BPC, C, HW], F32, kind="ExternalInput")
    wu_d = nc.dram_tensor("wu", [128, NG * HID], F32, kind="ExternalInput")
    wv_d = nc.dram_tensor("wv", [128, NG * HID], F32, kind="ExternalInput")
    w2_d = nc.dram_tensor("w2", [HID, 1], F32, kind="ExternalInput")
    b1_d = nc.dram_tensor("b1", [HID, 1], F32, kind="ExternalInput")
    b2_d = nc.dram_tensor("b2", [128, 1], F32, kind="ExternalInput")
    id_d = nc.dram_tensor("ident", [128, 128], F32, kind="ExternalInput")
    out_d = nc.dram_tensor("out", [BPC, C, HW], F32, kind="ExternalOutput")

    Relu = mybir.ActivationFunctionType.Relu
    Sigmoid = mybir.ActivationFunctionType.Sigmoid
    Copy = mybir.ActivationFunctionType.Copy
    CW = HW // NCH

    with tile_mod.TileContext(nc) as tc:
        with (
            tc.tile_pool(name="xt", bufs=XT_BUFS) as p_xt,
            tc.tile_pool(name="xtc", bufs=NCH) as p_chunk,
            tc.tile_pool(name="small", bufs=2) as p_small,
            tc.tile_pool(name="consts", bufs=1) as p_const,
            tc.tile_pool(name="ps_row", bufs=1, space="PSUM") as pp_row,
            tc.tile_pool(name="ps_bc", bufs=1, space="PSUM") as pp_bc,
            tc.tile_pool(name="ps_uv", bufs=1, space="PSUM") as pp_uv,
            tc.tile_pool(name="ps_a", bufs=1, space="PSUM") as pp_a,
        ):
            def load_sample(b):
                """Emit loads + incremental reduces; returns (xsum, tiles).

                tiles[g] is either a full [128, HW] tile, or (for the very
                last group of the last sample) a list of NCH chunk tiles —
                chunked so the final reduce finishes right after the last
                bytes land instead of a full 4096-wide reduce later.
                """
                xsum = p_small.tile([128, NG], F32, tag="xsum")
                tiles = []
                for g in range(NG):
                    if b == BPC - 1 and g == NG - 1:
                        tmp = p_small.tile([128, NCH], F32, tag="redtmp")
                        chunks = []
                        for cix in range(NCH):
                            xc = p_chunk.tile([128, CW], F32, tag="xtc")
                            nc.sync.dma_start(
                                out=xc[:],
                                in_=x_d[b, 128 * g : 128 * (g + 1),
                                        CW * cix : CW * (cix + 1)],
                            )
                            nc.vector.reduce_sum(
                                out=tmp[:, cix : cix + 1], in_=xc[:],
                                axis=mybir.AxisListType.X,
                            )
                            chunks.append(xc)
                        nc.vector.reduce_sum(
                            out=xsum[:, g : g + 1], in_=tmp[:],
                            axis=mybir.AxisListType.X,
                        )
                        tiles.append(chunks)
                    else:
                        xt = p_xt.tile([128, HW], F32, tag="xt")
                        nc.sync.dma_start(
                            out=xt[:], in_=x_d[b, 128 * g : 128 * (g + 1), :]
                        )
                        nc.vector.reduce_sum(
                            out=xsum[:, g : g + 1], in_=xt[:],
                            axis=mybir.AxisListType.X,
                        )
                        tiles.append(xt)
                return xsum, tiles

            # loads + reduces run one sample ahead of the gate/store stream
            pending = {0: load_sample(0)}

            # consts go via the gpsimd SWDGE queue so the SP ring is pure x-DMA
            ident_sb = p_const.tile([128, 128], F32, tag="ident")
            nc.gpsimd.dma_start(out=ident_sb[:], in_=id_d[:])
            wu_sb = p_const.tile([128, NG * HID], F32, tag="wu")
            nc.gpsimd.dma_start(out=wu_sb[:], in_=wu_d[:])
            wv_sb = p_const.tile([128, NG * HID], F32, tag="wv")
            nc.gpsimd.dma_start(out=wv_sb[:], in_=wv_d[:])
            w2_sb = p_const.tile([HID, 1], F32, tag="w2")
            nc.gpsimd.dma_start(out=w2_sb[:], in_=w2_d[:])
            b1_sb = p_const.tile([HID, 1], F32, tag="b1")
            nc.gpsimd.dma_start(out=b1_sb[:], in_=b1_d[:])
            b2_sb = p_const.tile([128, 1], F32, tag="b2")
            nc.gpsimd.dma_start(out=b2_sb[:], in_=b2_d[:])
            ones_sb = p_const.tile([1, HID], F32, tag="ones")
            nc.vector.memset(ones_sb[:], 1.0)

            for b in range(BPC):
                if b + 1 < BPC:
                    pending[b + 1] = load_sample(b + 1)
                xsum, tiles = pending.pop(b)
                last = b == BPC - 1

                # u = Wu^T @ xsum, v = Wv^T @ xsum   (accumulate over groups)
                u_ps = pp_uv.tile([HID, 1], F32, tag="u")
                v_ps = pp_uv.tile([HID, 1], F32, tag="v")
                for g in range(NG):
                    nc.tensor.matmul(
                        out=u_ps[:], lhsT=wu_sb[:, HID * g : HID * (g + 1)],
                        rhs=xsum[:, g : g + 1],
                        start=(g == 0), stop=(g == NG - 1),
                    )
                for g in range(NG):
                    nc.tensor.matmul(
                        out=v_ps[:], lhsT=wv_sb[:, HID * g : HID * (g + 1)],
                        rhs=xsum[:, g : g + 1],
                        start=(g == 0), stop=(g == NG - 1),
                    )
                ub_sb = p_small.tile([HID, 1], F32, tag="ub")
                nc.vector.tensor_add(out=ub_sb[:], in0=u_ps[:], in1=b1_sb[:])
                v_sb = p_small.tile([HID, 1], F32, tag="vsb")
                nc.vector.tensor_copy(out=v_sb[:], in_=v_ps[:])

                # per-group gate pipeline: each group's scale factor becomes
                # ready as early as possible (short last-gate tail)
                xrow_ps = pp_row.tile([1, C], F32, tag="xrow")
                xrow_sb = p_small.tile([1, C], F32, tag="xrowsb")
                xbc_ps = pp_bc.tile([HID, C], F32, tag="xbc")
                sT_sb = p_small.tile([HID, C], F32, tag="sT")
                a_ps = pp_a.tile([128, NG], F32, tag="aps")
                a_sb = p_small.tile([128, NG], F32, tag="asb")
                for g in range(NG):
                    sl = slice(128 * g, 128 * (g + 1))
                    # x_sum column -> row piece (tiny PE transpose), to SBUF
                    nc.tensor.transpose(
                        out=xrow_ps[0:1, sl], in_=xsum[:, g : g + 1],
                        identity=ident_sb[:],
                    )
                    nc.vector.tensor_copy(
                        out=xrow_sb[0:1, sl], in_=xrow_ps[0:1, sl]
                    )
                    # broadcast over HID partitions: ones[1,HID]^T @ row piece
                    nc.tensor.matmul(
                        out=xbc_ps[:, sl], lhsT=ones_sb[:],
                        rhs=xrow_sb[0:1, sl], start=True, stop=True,
                    )
                    # s^T[h, i] = relu(v[h] * xsum[i] + u[h] + b1[h])
                    nc.scalar.activation(
                        out=sT_sb[:, sl], in_=xbc_ps[:, sl], func=Relu,
                        bias=ub_sb[:], scale=v_sb[:],
                    )
                    # a_pre[i] = sum_h s^T[h, i] * w2[h]
                    nc.tensor.matmul(
                        out=a_ps[:, g : g + 1], lhsT=sT_sb[:, sl],
                        rhs=w2_sb[:], start=True, stop=True,
                    )
                    nc.scalar.activation(
                        out=a_sb[:, g : g + 1], in_=a_ps[:, g : g + 1],
                        func=Sigmoid, bias=b2_sb[:], scale=1.0,
                    )

                # scale in place and store.  Last sample alternates DVE/ACT
                # scales to compress the tail (safe there — no later reduces
                # to serialize behind on the DVE FIFO).
                def scale(tile_ap, a_col, on_dve):
                    if on_dve:
                        nc.vector.tensor_scalar_mul(
                            out=tile_ap, in0=tile_ap, scalar1=a_col
                        )
                    else:
                        nc.scalar.activation(
                            out=tile_ap, in_=tile_ap, func=Copy, scale=a_col
                        )

                for g in range(NG):
                    a_col = a_sb[:, g : g + 1]
                    if isinstance(tiles[g], list):
                        for cix, xc in enumerate(tiles[g]):
                            scale(xc[:], a_col, last and cix % 2 == 0)
                            nc.sync.dma_start(
                                out=x_dst(out_d, b, g, cix), in_=xc[:]
                            )
                    else:
                        scale(tiles[g][:], a_col, last and g % 2 == 0)
                        nc.sync.dma_start(
                            out=out_d[b, 128 * g : 128 * (g + 1), :],
                            in_=tiles[g][:],
                        )
    nc.compile()
    return nc


def x_dst(out_d, b, g, cix):
    CW = HW // NCH
    return out_d[b, 128 * g : 128 * (g + 1), CW * cix : CW * (cix + 1)]


_NC_CACHE = None


def _get_nc():
    global _NC_CACHE
    if _NC_CACHE is None:
        _NC_CACHE = _build_nc()
    return _NC_CACHE


def kernel(x, fc1_w, fc1_b, fc2_w, fc2_b):
    global LAST_RESULT
    x = np.ascontiguousarray(np.asarray(x, dtype=np.float32))
    fc1_w = np.asarray(fc1_w, dtype=np.float32)
    fc1_b = np.asarray(fc1_b, dtype=np.float32)
    fc2_w = np.asarray(fc2_w, dtype=np.float32)
    fc2_b = np.asarray(fc2_b, dtype=np.float32)

    # fold 1/HW (mean), the window-4 pooling and the affinity symmetry into
    # the fc1 weights; keep them as [k=channel-chunk, m] lhsT layouts.
    wu_full = fc1_w[:, :C].T / float(HW)                       # [C, HID]
    w1bc = fc1_w[:, C : C + 128] + fc1_w[:, C + 128 : C + 256]  # [HID, 128]
    wv_full = np.repeat(w1bc.T, 4, axis=0) / (4.0 * HW * HW)   # [C, HID]
    wu = np.ascontiguousarray(
        wu_full.reshape(NG, 128, HID).transpose(1, 0, 2).reshape(128, NG * HID),
        dtype=np.float32,
    )
    wv = np.ascontiguousarray(
        wv_full.reshape(NG, 128, HID).transpose(1, 0, 2).reshape(128, NG * HID),
        dtype=np.float32,
    )
    w2 = np.ascontiguousarray(fc2_w.reshape(1, HID).T, dtype=np.float32)  # [HID,1]
    b1 = np.ascontiguousarray(fc1_b.reshape(HID, 1), dtype=np.float32)
    b2 = np.full((128, 1), float(fc2_b.reshape(-1)[0]), dtype=np.float32)
    ident = np.eye(128, dtype=np.float32)

    xs = x.reshape(N_CORES, BPC, C, HW)
    in_maps = [
        {
            "x": xs[i],
            "wu": wu,
            "wv": wv,
            "w2": w2,
            "b1": b1,
            "b2": b2,
            "ident": ident,
        }
        for i in range(N_CORES)
    ]

    nc = _get_nc()
    res = None
    err = None
    for attempt in range(3):
        try:
            res = run_bass_kernel_spmd(nc, in_maps, list(range(N_CORES)))
            break
        except Exception as e:  # transient device errors: retry
            err = e
            import time

            time.sleep(2.0 * (attempt + 1))
    if res is None:
        raise err
    LAST_RESULT = res
    out = np.empty((B, C, H, W), dtype=np.float32)
    for i in range(N_CORES):
        out[i * BPC : (i + 1) * BPC] = res.results[i]["out"].reshape(BPC, C, H, W)
    return out
